# revision 22
# baseline (speedup 1.0000x reference)
"""Trainium2 Bass kernel for banded local attention (v2).

Reference (B=2, S=2048, D=512, H=8, dh=64, local_range=7):
  q = hs @ Wq, k = hs @ Wk (per-head slices)
  scores = q k^T / sqrt(dh); w = softmax(scores) * band; w /= sum(w) + 1e-6
  ctx = w @ hs                                  -> [B, H, S, D]

The band renormalization cancels the full-softmax denominator (up to the
1e-6*Z term, ~1e-4 relative), so only the 15-diagonal band of scores is
computed.

Sharding: core c = (batch c//4, S-quarter c%4) -> each core owns 512 rows
of one batch for ALL 8 heads.  Per-core inputs: hs^T window [512, 526]
(projections), per-tile hs windows (ctx), full Wq/Wk.  Row tiles of
T<=104 keep each ctx matmul a single K<=128-contraction matmul.

Scores are computed TRANSPOSED ([j, i]) so exp() output feeds the ctx
matmul directly as lhsT (no PE transposes).  Band mask is a 0/1 bf16
multiply on DVE.  Row sums come from an N=1 matmul against a ones
vector; the division happens on the host (output = raw ctx + sums in
bf16, host divides in f32).
"""

import numpy as np
import ml_dtypes

BF = ml_dtypes.bfloat16
S, D, H, DH = 2048, 512, 8, 64
NCORES = 8
ROWS = 512              # rows per core (S quarter)
WIN = 526               # core window: rows +/- 7 halo
T_LIST = [104, 104, 104, 104, 96]
TOFF = [0, 104, 208, 312, 416]
WIN_T = [t + 14 for t in T_LIST]

TRACE = False
WARMUP = False
LAST_RESULTS = None

_NC_CACHE = {}


def _build_nc():
    import concourse.bacc as bacc
    import concourse.mybir as mybir
    import concourse.tile as tile

    f32 = mybir.dt.float32
    bf16 = mybir.dt.bfloat16
    AF = mybir.ActivationFunctionType
    MUL = mybir.AluOpType.mult

    nc = bacc.Bacc("TRN2", target_bir_lowering=False, debug=False, num_devices=NCORES)

    hstw = nc.dram_tensor("hstw", [128, 4, WIN], bf16, kind="ExternalInput").ap()
    wpk = nc.dram_tensor("wpk", [8, 128, 4, 128], bf16, kind="ExternalInput").ap()
    hsw = nc.dram_tensor("hsw", [5, 128, D], bf16, kind="ExternalInput").ap()
    # identity (cols 0:128) + additive NEG band mask (cols 128:256)
    maskd = nc.dram_tensor("maskd", [128, 256], bf16, kind="ExternalInput").ap()
    onesd = nc.dram_tensor("onesd", [128, 8], bf16, kind="ExternalInput").ap()
    outd = nc.dram_tensor("out", [5, 2, 128, 4, 513], bf16, kind="ExternalOutput").ap()

    with tile.TileContext(nc) as tc:
        with (
            tc.tile_pool(name="const", bufs=1) as cpool,
            tc.tile_pool(name="ework", bufs=3) as epool,
            tc.tile_pool(name="outp", bufs=3) as opool,
            tc.tile_pool(name="psc", bufs=3, space="PSUM") as pscp,
            tc.tile_pool(name="pbig", bufs=2, space="PSUM") as pbigp,
        ):
            hstw_sb = cpool.tile([128, 4, WIN], bf16)
            w_sb = cpool.tile([128, 8, 4, 128], bf16)
            hsw_sb = cpool.tile([128, 5, D], bf16)
            mask_sb = cpool.tile([128, 256], bf16)
            ones_sb = cpool.tile([128, 8], bf16)
            qk_sb = cpool.tile([128, 8, WIN], bf16)  # slabs: q d-tiles 0..3, k d-tiles 0..3
            wz = cpool.tile([128, 512], bf16)

            # PE warm-up: the cost model runs the tensor engine at half clock
            # until it has been continuously busy for 3us; dummy matmuls on a
            # zero tile during the initial DMA window eat that ramp.
            nc.vector.memset(wz, 0.0)

            # ---- input DMAs (order tuned so the first projections and the
            # first band tiles are never waiting on a transfer) ----
            nc.sync.dma_start(out=hstw_sb[:, 0], in_=hstw[:, 0])
            nc.sync.dma_start(out=w_sb[:, 0], in_=wpk[0])
            nc.sync.dma_start(out=hstw_sb[:, 1], in_=hstw[:, 1])
            nc.sync.dma_start(out=hstw_sb[:, 2], in_=hstw[:, 2])
            nc.sync.dma_start(out=hstw_sb[:, 3], in_=hstw[:, 3])
            nc.sync.dma_start(out=w_sb[:, 4], in_=wpk[4])
            nc.sync.dma_start(out=w_sb[:, 1], in_=wpk[1])
            nc.sync.dma_start(out=w_sb[:, 5], in_=wpk[5])
            nc.sync.dma_start(out=mask_sb, in_=maskd)
            nc.sync.dma_start(out=ones_sb, in_=onesd)
            nc.sync.dma_start(out=hsw_sb, in_=hsw.rearrange("t p d -> p t d"))
            nc.sync.dma_start(out=w_sb[:, 2], in_=wpk[2])
            nc.sync.dma_start(out=w_sb[:, 6], in_=wpk[6])
            nc.sync.dma_start(out=w_sb[:, 3], in_=wpk[3])
            nc.sync.dma_start(out=w_sb[:, 7], in_=wpk[7])

            if WARMUP:
                # warm-up psum is read once by a dummy eviction into wz so the
                # BIR verifier sees a reader (unread PSUM upset the runtime)
                pwz = pbigp.tile([128, 1024], f32, tag="pbig")
                for i in range(6):
                    nc.tensor.matmul(pwz[:, 0:512], wz[:, 0:128], wz,
                                     start=(i == 0), stop=(i == 5))
                nc.vector.tensor_copy(wz[:, 0:512], pwz[:, 0:512])

            # GPSIMD cannot touch PSUM on TRN2, so evictions go to DVE/ACT
            # only; Pool earns its keep on the SBUF->SBUF mask multiplies.
            def _ev_dve(out, in_):
                nc.vector.tensor_copy(out, in_)

            def _ev_act(out, in_):
                nc.scalar.copy(out, in_)

            def emit_proj(slab, ev):
                ps = pbigp.tile([128, 1024], f32, tag="pbig")
                for cc in range(4):
                    nc.tensor.matmul(ps[:, 0:512], w_sb[:, slab, cc], hstw_sb[:, cc, 0:512],
                                     start=(cc == 0), stop=(cc == 3))
                for cc in range(4):
                    nc.tensor.matmul(ps[:, 512:WIN], w_sb[:, slab, cc], hstw_sb[:, cc, 512:WIN],
                                     start=(cc == 0), stop=(cc == 3))
                ev(qk_sb[:, slab], ps[:, 0:WIN])

            id_sb = mask_sb[:, 0:128]
            neg_sb = mask_sb[:, 128:256]

            def emit_half(t, half):
                T = T_LIST[t]
                win = WIN_T[t]
                toff = TOFF[t]
                psc = pscp.tile([128, 4, 128], f32, tag="psc")
                for hh in range(4):
                    h = 4 * half + hh
                    dt = h // 2
                    hp = slice(64 * (h % 2), 64 * (h % 2) + 64)
                    # additive NEG band mask via identity matmul opens each
                    # group with a K=128 base-0 matmul (HW needs this before
                    # the base-64 K=64 score matmuls share a psum tile);
                    # exp(score + NEG) == 0 outside the band.
                    nc.tensor.matmul(psc[0:win, hh, 0:T],
                                     id_sb[:, 0:win], neg_sb[:, 0:T],
                                     start=True, stop=False)
                    nc.tensor.matmul(psc[0:win, hh, 0:T],
                                     qk_sb[hp, 4 + dt, toff:toff + win],
                                     qk_sb[hp, dt, 7 + toff:7 + toff + T],
                                     start=False, stop=True)
                E = epool.tile([128, 4, 128], bf16, tag="E")
                nc.scalar.activation(E[0:win, :, 0:T], psc[0:win, :, 0:T], AF.Exp)
                o_sb = opool.tile([128, 4, 513], bf16, tag="o")
                for pair in range(2):
                    pctx = pbigp.tile([128, 1024], f32, tag="pbig")
                    for hh2 in range(2):
                        hh = 2 * pair + hh2
                        nc.tensor.matmul(pctx[0:T, 512 * hh2:512 * (hh2 + 1)],
                                         E[0:win, hh, 0:T], hsw_sb[0:win, t, :],
                                         start=True, stop=True)
                    pv = pctx[0:T, :].rearrange("p (g d) -> p g d", g=2)
                    ev = _ev_dve if pair == 0 else _ev_act
                    ev(o_sb[0:T, 2 * pair:2 * pair + 2, 0:512], pv)
                # band sums via N=1 matmul with the (edge-aware) ones column;
                # a zero K=128 preamble keeps the psum-tile write pattern in
                # the HW-proven shape (base-0 K=128 opens every group)
                for hh in range(4):
                    nc.tensor.matmul(psc[0:T, hh, 127:128],
                                     id_sb[:, 0:T], ones_sb[:, 5:6],
                                     start=True, stop=False)
                    nc.tensor.matmul(psc[0:T, hh, 127:128], E[0:win, hh, 0:T],
                                     ones_sb[0:win, t:t + 1],
                                     start=False, stop=True)
                nc.vector.tensor_copy(o_sb[0:T, :, 512:513], psc[0:T, :, 127:128])
                nc.sync.dma_start(out=outd[t, half, 0:T], in_=o_sb[0:T])

            # interleave: projections for d-tiles 0/1 unlock half 0 of every
            # tile; remaining projections stream between band-tile jobs
            emit_proj(0, _ev_act)   # q d-tile 0
            emit_proj(4, _ev_dve)   # k d-tile 0
            emit_proj(1, _ev_act)   # q d-tile 1
            emit_proj(5, _ev_dve)   # k d-tile 1
            emit_half(0, 0)
            emit_proj(2, _ev_act)
            emit_half(1, 0)
            emit_proj(6, _ev_dve)
            emit_half(2, 0)
            emit_proj(3, _ev_act)
            emit_half(3, 0)
            emit_proj(7, _ev_dve)
            emit_half(4, 0)
            emit_half(0, 1)
            emit_half(1, 1)
            emit_half(2, 1)
            emit_half(3, 1)
            emit_half(4, 1)

    nc.compile()
    return nc


def _get_nc():
    if "nc" not in _NC_CACHE:
        _NC_CACHE["nc"] = _build_nc()
    return _NC_CACHE["nc"]


def _prep_core(hs_f32, wq_s, wk_s, b, qq):
    """Host-side input prep for core (b, qq). hs_f32: [B, S, D] float32;
    wq_s/wk_s: [D, D] float32 (wq pre-scaled by 1/sqrt(dh))."""
    R0 = ROWS * qq
    g = np.arange(R0 - 7, R0 + ROWS + 7)
    valid = (g >= 0) & (g < S)
    hs_slice = np.zeros((WIN, D), np.float32)
    hs_slice[valid] = hs_f32[b, g[valid]]
    hstw = np.ascontiguousarray(
        hs_slice.T.reshape(4, 128, WIN).transpose(1, 0, 2)).astype(BF)

    hsw = np.zeros((5, 128, D), np.float32)
    for t in range(5):
        g2 = np.arange(R0 + TOFF[t] - 7, R0 + TOFF[t] - 7 + 128)
        v2 = (g2 >= 0) & (g2 < S)
        hsw[t, v2] = hs_f32[b, g2[v2]]
    hsw = hsw.astype(BF)

    slabs = []
    for w in (wq_s, wk_s):
        for j in range(4):
            slabs.append(w[:, 128 * j:128 * (j + 1)].reshape(4, 128, 128).transpose(1, 0, 2))
    wpk = np.ascontiguousarray(np.stack(slabs, axis=0)).astype(BF)

    ones = np.ones((128, 8), np.float32)
    if qq == 0:
        ones[0:7, 0] = 0.0
    if qq == 3:
        ones[103:, 4] = 0.0
    ones[:, 5] = 0.0  # zeros column: K=128 preamble for the sums matmuls
    ones = ones.astype(BF)
    return {"hstw": hstw, "wpk": wpk, "hsw": np.ascontiguousarray(hsw), "onesd": ones}


def kernel(hidden_states, Wq, Wk):
    global LAST_RESULTS
    from concourse import bass_utils

    hs_f32 = np.asarray(hidden_states, dtype=np.float32)
    wq_s = np.asarray(Wq, dtype=np.float32) * (1.0 / (DH ** 0.5))
    wk_s = np.asarray(Wk, dtype=np.float32)

    p = np.arange(128)[:, None]
    f = np.arange(128)[None, :]
    negmask = np.where((p - f >= 0) & (p - f <= 14), 0.0, -10000.0).astype(np.float32)
    maskd = np.concatenate([np.eye(128, dtype=np.float32), negmask], axis=1).astype(BF)

    in_maps = []
    for c in range(NCORES):
        m = _prep_core(hs_f32, wq_s, wk_s, c // 4, c % 4)
        m["maskd"] = maskd
        in_maps.append(m)

    nc = _get_nc()
    res = bass_utils.run_bass_kernel_spmd(
        nc, in_maps, core_ids=list(range(NCORES)), trace=TRACE,
    )
    LAST_RESULTS = res

    out = np.empty((2, H, S, D), np.float32)
    for c in range(NCORES):
        b, qq = c // 4, c % 4
        R0 = ROWS * qq
        arr = np.asarray(res.results[c]["out"]).astype(np.float32)  # [5,2,128,4,513]
        for t in range(5):
            T = T_LIST[t]
            for half in range(2):
                blk = arr[t, half, 0:T]                 # [T, 4, 513]
                ctx = blk[:, :, 0:512]
                sm = blk[:, :, 512]
                out[b, 4 * half:4 * half + 4, R0 + TOFF[t]:R0 + TOFF[t] + T, :] = \
                    (ctx / sm[:, :, None]).transpose(1, 0, 2)
    return out


# revision 37
# speedup vs baseline: 1.2131x; 1.2131x over previous
"""Trainium2 Bass kernel for banded local attention (v2).

Reference (B=2, S=2048, D=512, H=8, dh=64, local_range=7):
  q = hs @ Wq, k = hs @ Wk (per-head slices)
  scores = q k^T / sqrt(dh); w = softmax(scores) * band; w /= sum(w) + 1e-6
  ctx = w @ hs                                  -> [B, H, S, D]

The band renormalization cancels the full-softmax denominator (up to the
1e-6*Z term, ~1e-4 relative), so only the 15-diagonal band of scores is
computed.

Sharding: core c = (batch c//4, S-quarter c%4) -> each core owns 512 rows
of one batch for ALL 8 heads.  Per-core inputs: hs^T window [512, 526]
(projections), per-tile hs windows (ctx), full Wq/Wk.  Row tiles of
T<=104 keep each ctx matmul a single K<=128-contraction matmul.

Scores are computed TRANSPOSED ([j, i]) so exp() output feeds the ctx
matmul directly as lhsT (no PE transposes).  Band mask is a 0/1 bf16
multiply on DVE.  Row sums come from an N=1 matmul against a ones
vector; the division happens on the host (output = raw ctx + sums in
bf16, host divides in f32).
"""

import numpy as np
import ml_dtypes

BF = ml_dtypes.bfloat16
S, D, H, DH = 2048, 512, 8, 64
NCORES = 8
ROWS = 512              # rows per core (S quarter)
WIN = 526               # core window: rows +/- 7 halo
T_LIST = [104, 104, 104, 104, 96]
TOFF = [0, 104, 208, 312, 416]
WIN_T = [t + 14 for t in T_LIST]

TRACE = False
WARMUP = True
LAST_RESULTS = None

_NC_CACHE = {}


def _build_nc():
    import concourse.bacc as bacc
    import concourse.mybir as mybir
    import concourse.tile as tile

    f32 = mybir.dt.float32
    bf16 = mybir.dt.bfloat16
    AF = mybir.ActivationFunctionType
    MUL = mybir.AluOpType.mult

    nc = bacc.Bacc("TRN2", target_bir_lowering=False, debug=False, num_devices=NCORES)

    hstw = nc.dram_tensor("hstw", [128, 4, WIN], bf16, kind="ExternalInput").ap()
    wpk = nc.dram_tensor("wpk", [8, 128, 4, 128], bf16, kind="ExternalInput").ap()
    hsw = nc.dram_tensor("hsw", [5, 128, D], bf16, kind="ExternalInput").ap()
    # identity (cols 0:128) + additive NEG band mask (cols 128:256)
    maskd = nc.dram_tensor("maskd", [128, 256], bf16, kind="ExternalInput").ap()
    onesd = nc.dram_tensor("onesd", [128, 8], bf16, kind="ExternalInput").ap()
    # ctx^T, packed per job: [d%128, job-col-range of 16*T (4 heads x 4
    # d-chunks x T rows)] — exactly 4 MiB, no padding
    outd = nc.dram_tensor("out", [128, 16384], bf16, kind="ExternalOutput").ap()
    sumsd = nc.dram_tensor("sums", [128, 5, 8], f32, kind="ExternalOutput").ap()

    with tile.TileContext(nc) as tc:
        with (
            tc.tile_pool(name="const", bufs=1) as cpool,
            tc.tile_pool(name="ework", bufs=3) as epool,
            tc.tile_pool(name="outp", bufs=3) as opool,
            tc.tile_pool(name="psc", bufs=2, space="PSUM") as pscp,
            tc.tile_pool(name="pbig", bufs=3, space="PSUM") as pbigp,
        ):
            hstw_sb = cpool.tile([128, 4, WIN], bf16)
            w_sb = cpool.tile([128, 8, 4, 128], bf16)
            hsw_sb = cpool.tile([128, 5, D], bf16)
            mask_sb = cpool.tile([128, 256], bf16)
            ones_sb = cpool.tile([128, 8], bf16)
            qk_sb = cpool.tile([128, 8, WIN], bf16)  # slabs: q d-tiles 0..3, k d-tiles 0..3
            s_all = cpool.tile([128, 5, 8], f32)     # per-row band sums
            wz = cpool.tile([128, 512], bf16)
            nc.vector.memset(s_all, 1.0)

            # PE warm-up: the cost model runs the tensor engine at half clock
            # until it has been continuously busy for 3us; dummy matmuls on a
            # zero tile during the initial DMA window eat that ramp.
            nc.vector.memset(wz, 0.0)

            # ---- input DMAs (order tuned so the first projections and the
            # first band tiles are never waiting on a transfer) ----
            nc.sync.dma_start(out=hstw_sb[:, 0], in_=hstw[:, 0])
            nc.sync.dma_start(out=w_sb[:, 0], in_=wpk[0])
            nc.sync.dma_start(out=hstw_sb[:, 1], in_=hstw[:, 1])
            nc.sync.dma_start(out=hstw_sb[:, 2], in_=hstw[:, 2])
            nc.sync.dma_start(out=hstw_sb[:, 3], in_=hstw[:, 3])
            nc.sync.dma_start(out=w_sb[:, 4], in_=wpk[4])
            nc.sync.dma_start(out=w_sb[:, 1], in_=wpk[1])
            nc.sync.dma_start(out=w_sb[:, 5], in_=wpk[5])
            nc.sync.dma_start(out=mask_sb, in_=maskd)
            nc.sync.dma_start(out=ones_sb, in_=onesd)
            nc.sync.dma_start(out=hsw_sb, in_=hsw.rearrange("t p d -> p t d"))
            nc.sync.dma_start(out=w_sb[:, 2], in_=wpk[2])
            nc.sync.dma_start(out=w_sb[:, 6], in_=wpk[6])
            nc.sync.dma_start(out=w_sb[:, 3], in_=wpk[3])
            nc.sync.dma_start(out=w_sb[:, 7], in_=wpk[7])

            if WARMUP:
                # warm-up psum is read once by a dummy eviction into wz so the
                # BIR verifier sees a reader (unread PSUM upset the runtime)
                pwz = pbigp.tile([128, 1024], f32, tag="pbig")
                for i in range(6):
                    nc.tensor.matmul(pwz[:, 0:512], wz[:, 0:128], wz,
                                     start=(i == 0), stop=(i == 5))
                nc.vector.tensor_copy(wz[:, 0:512], pwz[:, 0:512])

            # GPSIMD cannot touch PSUM on TRN2, so evictions go to DVE/ACT
            # only; Pool earns its keep on the SBUF->SBUF mask multiplies.
            def _ev_dve(out, in_):
                nc.vector.tensor_copy(out, in_)

            def _ev_act(out, in_):
                nc.scalar.copy(out, in_)

            def emit_proj(slab, ev):
                ps = pbigp.tile([128, 1024], f32, tag="pbig")
                for cc in range(4):
                    nc.tensor.matmul(ps[:, 0:512], w_sb[:, slab, cc], hstw_sb[:, cc, 0:512],
                                     start=(cc == 0), stop=(cc == 3))
                for cc in range(4):
                    nc.tensor.matmul(ps[:, 512:WIN], w_sb[:, slab, cc], hstw_sb[:, cc, 512:WIN],
                                     start=(cc == 0), stop=(cc == 3))
                ev(qk_sb[:, slab], ps[:, 0:WIN])

            id_sb = mask_sb[:, 0:128]
            neg_sb = mask_sb[:, 128:256]

            def emit_scores(t, half):
                T = T_LIST[t]
                win = WIN_T[t]
                toff = TOFF[t]
                psc = pscp.tile([128, 4, 128], f32, tag="psc")
                for hh in range(4):
                    h = 4 * half + hh
                    dt = h // 2
                    hp = slice(64 * (h % 2), 64 * (h % 2) + 64)
                    # additive NEG band mask via identity matmul opens each
                    # group with a K=128 base-0 matmul (HW needs this before
                    # the base-64 K=64 score matmuls share a psum tile);
                    # exp(score + NEG) == 0 outside the band.
                    nc.tensor.matmul(psc[0:win, hh, 0:T],
                                     id_sb[:, 0:win], neg_sb[:, 0:T],
                                     start=True, stop=False)
                    nc.tensor.matmul(psc[0:win, hh, 0:T],
                                     qk_sb[hp, 4 + dt, toff:toff + win],
                                     qk_sb[hp, dt, 7 + toff:7 + toff + T],
                                     start=False, stop=True)
                return psc

            job_idx = [0]

            def emit_exp(t, half, psc):
                T = T_LIST[t]
                win = WIN_T[t]
                E = epool.tile([128, 4, 128], bf16, tag="E")
                nc.scalar.activation(E[0:win, :, 0:T], psc[0:win, :, 0:T], AF.Exp)
                return E

            def emit_rest(t, half, psc, E, psc_next, ocol):
                T = T_LIST[t]
                win = WIN_T[t]
                o_sb = opool.tile([128, 16 * 104], bf16, tag="o")
                # band sums via N=1 matmul with the (edge-aware) ones column;
                # a zero K=128 preamble keeps the psum-tile write pattern in
                # the HW-proven shape (base-0 K=128 opens every group).
                # Sums land in the NEXT job's scores psum (col 126) so this
                # job's psc is freed by its exp -> psc pool stays 2-deep.
                for hh in range(4):
                    nc.tensor.matmul(psc_next[0:T, hh, 126:127],
                                     id_sb[:, 0:T], ones_sb[:, 5:6],
                                     start=True, stop=False)
                    nc.tensor.matmul(psc_next[0:T, hh, 126:127], E[0:win, hh, 0:T],
                                     ones_sb[0:win, t:t + 1],
                                     start=False, stop=True)
                nc.vector.tensor_copy(
                    s_all[0:T, t, 4 * half:4 * half + 4],
                    psc_next[0:T, :, 126:127].rearrange("p h one -> p (h one)"))
                j = job_idx[0]
                job_idx[0] += 1
                # ctx^T: hsw d-chunk stationary, E moving (N=T, not 512)
                for pair in range(2):
                    pct = pbigp.tile([128, 8, 128], f32, tag="pbig")
                    for hh2 in range(2):
                        hh = 2 * pair + hh2
                        for dc in range(4):
                            nc.tensor.matmul(pct[:, 4 * hh2 + dc, 0:T],
                                             hsw_sb[0:win, t, 128 * dc:128 * (dc + 1)],
                                             E[0:win, hh, 0:T],
                                             start=True, stop=True)
                    ov = o_sb[:, pair * 8 * T:(pair + 1) * 8 * T].rearrange(
                        "p (s i) -> p s i", s=8)
                    ev = _ev_dve if (pair == 0 or j == 4) else _ev_act
                    ev(ov, pct[:, :, 0:T])
                nc.sync.dma_start(out=outd[:, ocol:ocol + 16 * T],
                                  in_=o_sb[:, 0:16 * T])

            # Software pipeline: scores of job i+1 are queued on the PE
            # BEFORE job i's ctx matmuls, so the PE never head-of-line
            # blocks waiting for job i's exp on ACT.  Remaining projection
            # slabs stream between early jobs (their heads come later).
            jobs = [(t, 0) for t in range(5)] + [(t, 1) for t in range(5)]
            emit_proj(0, _ev_act)   # q d-tile 0
            emit_proj(4, _ev_dve)   # k d-tile 0
            emit_proj(1, _ev_act)   # q d-tile 1
            emit_proj(5, _ev_dve)   # k d-tile 1
            projs_left = [(2, _ev_act), (6, _ev_dve), (3, _ev_act), (7, _ev_dve)]
            # Three-stage pipeline A(scores) / E(exp) / R(ctx+evicts+dma):
            # every exp is queued on ACT ahead of the previous job's ACT
            # eviction, so ACT head-of-line blocking never gates the PE.
            n = len(jobs)
            ocols = []
            c0 = 0
            for t, half in jobs:
                ocols.append(c0)
                c0 += 16 * T_LIST[t]
            state = [None] * n  # (psc, E)
            for i in range(n):
                t, half = jobs[i]
                state[i] = [emit_scores(t, half), None]
                if i >= 1:
                    tp, hp_ = jobs[i - 1]
                    state[i - 1][1] = emit_exp(tp, hp_, state[i - 1][0])
                if i >= 2:
                    tp, hp_ = jobs[i - 2]
                    nxt = state[i - 1][0]
                    emit_rest(tp, hp_, state[i - 2][0], state[i - 2][1], nxt,
                              ocols[i - 2])
                if projs_left and i < 4:
                    emit_proj(*projs_left.pop(0))
            state[n - 1][1] = emit_exp(*jobs[n - 1], state[n - 1][0])
            emit_rest(*jobs[n - 2], state[n - 2][0], state[n - 2][1],
                      state[n - 1][0], ocols[n - 2])
            # bulk sums (jobs 0-8) can ship before the final job finishes
            nc.sync.dma_start(out=sumsd, in_=s_all)
            emit_rest(*jobs[n - 1], state[n - 1][0], state[n - 1][1],
                      state[n - 1][0], ocols[n - 1])
            # last job's four sums values go out in a tiny trailing DMA
            nc.sync.dma_start(out=sumsd[:, 4, 4:8], in_=s_all[:, 4, 4:8])

    nc.compile()
    return nc


def _get_nc():
    if "nc" not in _NC_CACHE:
        _NC_CACHE["nc"] = _build_nc()
    return _NC_CACHE["nc"]


def _prep_core(hs_f32, wq_s, wk_s, b, qq):
    """Host-side input prep for core (b, qq). hs_f32: [B, S, D] float32;
    wq_s/wk_s: [D, D] float32 (wq pre-scaled by 1/sqrt(dh))."""
    R0 = ROWS * qq
    g = np.arange(R0 - 7, R0 + ROWS + 7)
    valid = (g >= 0) & (g < S)
    hs_slice = np.zeros((WIN, D), np.float32)
    hs_slice[valid] = hs_f32[b, g[valid]]
    hstw = np.ascontiguousarray(
        hs_slice.T.reshape(4, 128, WIN).transpose(1, 0, 2)).astype(BF)

    hsw = np.zeros((5, 128, D), np.float32)
    for t in range(5):
        g2 = np.arange(R0 + TOFF[t] - 7, R0 + TOFF[t] - 7 + 128)
        v2 = (g2 >= 0) & (g2 < S)
        hsw[t, v2] = hs_f32[b, g2[v2]]
    hsw = hsw.astype(BF)

    slabs = []
    for w in (wq_s, wk_s):
        for j in range(4):
            slabs.append(w[:, 128 * j:128 * (j + 1)].reshape(4, 128, 128).transpose(1, 0, 2))
    wpk = np.ascontiguousarray(np.stack(slabs, axis=0)).astype(BF)

    ones = np.ones((128, 8), np.float32)
    if qq == 0:
        ones[0:7, 0] = 0.0
    if qq == 3:
        ones[103:, 4] = 0.0
    ones[:, 5] = 0.0  # zeros column: K=128 preamble for the sums matmuls
    ones = ones.astype(BF)
    return {"hstw": hstw, "wpk": wpk, "hsw": np.ascontiguousarray(hsw), "onesd": ones}


def kernel(hidden_states, Wq, Wk):
    global LAST_RESULTS
    from concourse import bass_utils

    hs_f32 = np.asarray(hidden_states, dtype=np.float32)
    wq_s = np.asarray(Wq, dtype=np.float32) * (1.0 / (DH ** 0.5))
    wk_s = np.asarray(Wk, dtype=np.float32)

    p = np.arange(128)[:, None]
    f = np.arange(128)[None, :]
    negmask = np.where((p - f >= 0) & (p - f <= 14), 0.0, -10000.0).astype(np.float32)
    maskd = np.concatenate([np.eye(128, dtype=np.float32), negmask], axis=1).astype(BF)

    in_maps = []
    for c in range(NCORES):
        m = _prep_core(hs_f32, wq_s, wk_s, c // 4, c % 4)
        m["maskd"] = maskd
        in_maps.append(m)

    nc = _get_nc()
    res = bass_utils.run_bass_kernel_spmd(
        nc, in_maps, core_ids=list(range(NCORES)), trace=TRACE,
    )
    LAST_RESULTS = res

    out = np.empty((2, H, S, D), np.float32)
    jobs = [(t, 0) for t in range(5)] + [(t, 1) for t in range(5)]
    for c in range(NCORES):
        b, qq = c // 4, c % 4
        R0 = ROWS * qq
        flat = np.asarray(res.results[c]["out"]).astype(np.float32)  # [128, 16384]
        sums = np.asarray(res.results[c]["sums"])                    # [128, 5, 8] f32
        off = 0
        for (t, half) in jobs:
            T = T_LIST[t]
            blk = flat[:, off:off + 16 * T].reshape(128, 2, 2, 4, T)
            off += 16 * T
            # blk[p, pair, hh2, dc, i] -> ctx[pair, hh2, i, dc*128+p]
            ctx = blk.transpose(1, 2, 4, 3, 0).reshape(2, 2, T, D)
            rows = slice(R0 + TOFF[t], R0 + TOFF[t] + T)
            for pair in range(2):
                for hh2 in range(2):
                    h = 4 * half + 2 * pair + hh2
                    sm = sums[0:T, t, 4 * half + 2 * pair + hh2]
                    out[b, h, rows, :] = ctx[pair, hh2] / sm[:, None]
    return out


# revision 67
# speedup vs baseline: 1.2983x; 1.0702x over previous
"""Trainium2 Bass kernel for banded local attention (v2).

Reference (B=2, S=2048, D=512, H=8, dh=64, local_range=7):
  q = hs @ Wq, k = hs @ Wk (per-head slices)
  scores = q k^T / sqrt(dh); w = softmax(scores) * band; w /= sum(w) + 1e-6
  ctx = w @ hs                                  -> [B, H, S, D]

The band renormalization cancels the full-softmax denominator (up to the
1e-6*Z term, ~1e-4 relative), so only the 15-diagonal band of scores is
computed.

Sharding: core c = (batch c//4, S-quarter c%4) -> each core owns 512 rows
of one batch for ALL 8 heads.  Per-core inputs: hs^T window [512, 526]
(projections), per-tile hs windows (ctx), full Wq/Wk.  Row tiles of
T<=104 keep each ctx matmul a single K<=128-contraction matmul.

Scores are computed TRANSPOSED ([j, i]) so exp() output feeds the ctx
matmul directly as lhsT (no PE transposes).  Band mask is a 0/1 bf16
multiply on DVE.  Row sums come from an N=1 matmul against a ones
vector; the division happens on the host (output = raw ctx + sums in
bf16, host divides in f32).
"""

import numpy as np
import ml_dtypes

BF = ml_dtypes.bfloat16
S, D, H, DH = 2048, 512, 8, 64
NCORES = 8
ROWS = 512              # rows per core (S quarter)
WIN = 526               # core window: rows +/- 7 halo
T_LIST = [104, 104, 104, 104, 96]
TOFF = [0, 104, 208, 312, 416]
WIN_T = [t + 14 for t in T_LIST]

TRACE = False
WARMUP = True
LAST_RESULTS = None

_NC_CACHE = {}


def _build_nc():
    import concourse.bacc as bacc
    import concourse.mybir as mybir
    import concourse.tile as tile

    f32 = mybir.dt.float32
    bf16 = mybir.dt.bfloat16
    AF = mybir.ActivationFunctionType
    MUL = mybir.AluOpType.mult

    nc = bacc.Bacc("TRN2", target_bir_lowering=False, debug=False, num_devices=NCORES)

    hstw = nc.dram_tensor("hstw", [128, 4, WIN], bf16, kind="ExternalInput").ap()
    wpk = nc.dram_tensor("wpk", [8, 128, 4, 128], bf16, kind="ExternalInput").ap()
    hsw = nc.dram_tensor("hsw", [6, 128, D], bf16, kind="ExternalInput").ap()
    # identity (cols 0:128) + additive NEG band mask (cols 128:256)
    maskd = nc.dram_tensor("maskd", [128, 256], bf16, kind="ExternalInput").ap()
    onesd = nc.dram_tensor("onesd", [128, 8], bf16, kind="ExternalInput").ap()
    # ctx^T, packed per job: [d%128, job-col-range of 16*T (4 heads x 4
    # d-chunks x T rows)] — exactly 4 MiB, no padding
    outd = nc.dram_tensor("out", [128, 16384], bf16, kind="ExternalOutput").ap()
    sumsd = nc.dram_tensor("sums", [128, 6, 8], f32, kind="ExternalOutput").ap()

    with tile.TileContext(nc) as tc:
        with (
            tc.tile_pool(name="const", bufs=1) as cpool,
            tc.tile_pool(name="ework", bufs=3) as epool,
            tc.tile_pool(name="outp", bufs=3) as opool,
            tc.tile_pool(name="psc", bufs=2, space="PSUM") as pscp,
            tc.tile_pool(name="pbig", bufs=3, space="PSUM") as pbigp,
        ):
            hstw_sb = cpool.tile([128, 4, WIN], bf16)
            w_sb = cpool.tile([128, 8, 4, 128], bf16)
            hsw_sb = cpool.tile([128, 6, D], bf16)
            mask_sb = cpool.tile([128, 256], bf16)
            ones_sb = cpool.tile([128, 8], bf16)
            qk_sb = cpool.tile([128, 8, WIN], bf16)  # slabs: q d-tiles 0..3, k d-tiles 0..3
            s_all = cpool.tile([128, 6, 8], f32)     # per-row band sums
            wz = cpool.tile([128, 512], bf16)
            nc.vector.memset(s_all, 1.0)

            # PE warm-up: the cost model runs the tensor engine at half clock
            # until it has been continuously busy for 3us; dummy matmuls on a
            # zero tile during the initial DMA window eat that ramp.
            nc.vector.memset(wz, 0.0)

            # ---- input DMAs (order tuned so the first projections and the
            # first band tiles are never waiting on a transfer) ----
            # W slabs are packed interleaved on the host: [q0,k0,q1,k1,...]
            # so one DMA delivers a (q,k) d-tile pair
            nc.sync.dma_start(out=hstw_sb[:, 0:2], in_=hstw[:, 0:2])
            nc.sync.dma_start(out=w_sb[:, 0:2],
                              in_=wpk[0:2].rearrange("s p c d -> p s c d"))
            nc.sync.dma_start(out=hstw_sb[:, 2:4], in_=hstw[:, 2:4])
            nc.sync.dma_start(out=mask_sb, in_=maskd)
            nc.sync.dma_start(out=ones_sb, in_=onesd)
            nc.sync.dma_start(out=w_sb[:, 2:4],
                              in_=wpk[2:4].rearrange("s p c d -> p s c d"))
            nc.sync.dma_start(out=hsw_sb[:, 0], in_=hsw[0])
            nc.sync.dma_start(out=w_sb[:, 4:6],
                              in_=wpk[4:6].rearrange("s p c d -> p s c d"))
            nc.sync.dma_start(out=w_sb[:, 6:8],
                              in_=wpk[6:8].rearrange("s p c d -> p s c d"))
            nc.sync.dma_start(out=hsw_sb[:, 1:6],
                              in_=hsw[1:6].rearrange("t p d -> p t d"))

            if WARMUP:
                # warm-up psum is read once by a dummy eviction into wz so the
                # BIR verifier sees a reader (unread PSUM upset the runtime)
                pwz = pbigp.tile([128, 1024], f32, tag="pbig")
                for i in range(6):
                    nc.tensor.matmul(pwz[:, 0:512], wz[:, 0:128], wz,
                                     start=(i == 0), stop=(i == 5))
                nc.vector.tensor_copy(wz[:, 0:512], pwz[:, 0:512])

            # GPSIMD cannot touch PSUM on TRN2, so evictions go to DVE/ACT
            # only; Pool earns its keep on the SBUF->SBUF mask multiplies.
            def _ev_dve(out, in_):
                nc.vector.tensor_copy(out, in_)

            def _ev_act(out, in_):
                nc.scalar.copy(out, in_)

            def emit_proj(slab, ev):
                ps = pbigp.tile([128, 1024], f32, tag="pbig")
                for cc in range(4):
                    nc.tensor.matmul(ps[:, 0:512], w_sb[:, slab, cc], hstw_sb[:, cc, 0:512],
                                     start=(cc == 0), stop=(cc == 3))
                for cc in range(4):
                    nc.tensor.matmul(ps[:, 512:WIN], w_sb[:, slab, cc], hstw_sb[:, cc, 512:WIN],
                                     start=(cc == 0), stop=(cc == 3))
                ev(qk_sb[:, slab], ps[:, 0:WIN])

            id_sb = mask_sb[:, 0:128]
            neg_sb = mask_sb[:, 128:256]

            def emit_scores(jb, half):
                toff, T, win, hslot, onecol, scol = jb
                psc = pscp.tile([128, 4, 128], f32, tag="psc")
                for hh in range(4):
                    h = 4 * half + hh
                    dt = h // 2
                    hp = slice(64 * (h % 2), 64 * (h % 2) + 64)
                    # additive NEG band mask via identity matmul opens each
                    # group with a K=128 base-0 matmul (HW needs this before
                    # the base-64 K=64 score matmuls share a psum tile);
                    # exp(score + NEG) == 0 outside the band.
                    nc.tensor.matmul(psc[0:win, hh, 0:T],
                                     id_sb[:, 0:win], neg_sb[:, 0:T],
                                     start=True, stop=False)
                    nc.tensor.matmul(psc[0:win, hh, 0:T],
                                     qk_sb[hp, 2 * dt + 1, toff:toff + win],
                                     qk_sb[hp, 2 * dt, 7 + toff:7 + toff + T],
                                     start=False, stop=True)
                return psc

            job_idx = [0]

            def emit_exp(jb, half, psc):
                toff, T, win, hslot, onecol, scol = jb
                E = epool.tile([128, 4, 128], bf16, tag="E")
                nc.scalar.activation(E[0:win, :, 0:T], psc[0:win, :, 0:T], AF.Exp)
                return E

            def emit_rest(jb, half, psc, E, psc_next, ocol, fine_dma=False):
                toff, T, win, hslot, onecol, scol = jb
                o_sb = opool.tile([128, 16 * 104], bf16, tag="o")
                # band sums via N=1 matmul with the (edge-aware) ones column;
                # a zero K=128 preamble keeps the psum-tile write pattern in
                # the HW-proven shape (base-0 K=128 opens every group).
                # Sums land in the NEXT job's scores psum (col 126) so this
                # job's psc is freed by its exp -> psc pool stays 2-deep.
                for hh in range(4):
                    nc.tensor.matmul(psc_next[0:T, hh, 126:127],
                                     id_sb[:, 0:T], ones_sb[:, 5:6],
                                     start=True, stop=False)
                    nc.tensor.matmul(psc_next[0:T, hh, 126:127], E[0:win, hh, 0:T],
                                     ones_sb[0:win, onecol:onecol + 1],
                                     start=False, stop=True)
                nc.vector.tensor_copy(
                    s_all[0:T, scol, 4 * half:4 * half + 4],
                    psc_next[0:T, :, 126:127].rearrange("p h one -> p (h one)"))
                j = job_idx[0]
                job_idx[0] += 1
                # ctx^T: hsw d-chunk stationary, E moving (N=T, not 512)
                for pair in range(2):
                    pct = pbigp.tile([128, 8, 128], f32, tag="pbig")
                    for hh2 in range(2):
                        hh = 2 * pair + hh2
                        for dc in range(4):
                            nc.tensor.matmul(pct[:, 4 * hh2 + dc, 0:T],
                                             hsw_sb[0:win, hslot, 128 * dc:128 * (dc + 1)],
                                             E[0:win, hh, 0:T],
                                             start=True, stop=True)
                    ov = o_sb[:, pair * 8 * T:(pair + 1) * 8 * T].rearrange(
                        "p (s i) -> p s i", s=8)
                    ev = _ev_dve if pair == 0 else _ev_act
                    ev(ov, pct[:, :, 0:T])
                    if fine_dma:
                        # drain tail: ship each pair as soon as it's evicted
                        cc0 = ocol + pair * 8 * T
                        nc.sync.dma_start(out=outd[:, cc0:cc0 + 8 * T],
                                          in_=o_sb[:, pair * 8 * T:(pair + 1) * 8 * T])
                if not fine_dma:
                    nc.sync.dma_start(out=outd[:, ocol:ocol + 16 * T],
                                      in_=o_sb[:, 0:16 * T])

            # Software pipeline: scores of job i+1 are queued on the PE
            # BEFORE job i's ctx matmuls, so the PE never head-of-line
            # blocks waiting for job i's exp on ACT.  Remaining projection
            # slabs stream between early jobs (their heads come later).
            # job tuple: (toff, T, win, hsw-slot, ones-col, sums-col)
            jdef = [(TOFF[t], T_LIST[t], WIN_T[t], t, t, t) for t in range(5)]
            jobs = [(jdef[t], 0) for t in range(5)] + [(jdef[t], 1) for t in range(5)]
            emit_proj(0, _ev_act)   # q d-tile 0
            emit_proj(1, _ev_dve)   # k d-tile 0
            emit_proj(2, _ev_act)   # q d-tile 1
            emit_proj(3, _ev_dve)   # k d-tile 1
            projs_left = [(4, _ev_act), (5, _ev_dve), (6, _ev_act), (7, _ev_dve)]
            # Three-stage pipeline A(scores) / E(exp) / R(ctx+evicts+dma):
            # every exp is queued on ACT ahead of the previous job's ACT
            # eviction, so ACT head-of-line blocking never gates the PE.
            n = len(jobs)
            ocols = []
            c0 = 0
            for jb, half in jobs:
                ocols.append(c0)
                c0 += 16 * jb[1]
            state = [None] * n  # (psc, E)
            for i in range(n):
                jb, half = jobs[i]
                psc_i = emit_scores(jb, half)
                state[i] = [psc_i, emit_exp(jb, half, psc_i)]
                if i >= 2:
                    jp, hp_ = jobs[i - 2]
                    emit_rest(jp, hp_, state[i - 2][0], state[i - 2][1],
                              state[i - 1][0], ocols[i - 2])
                if projs_left and i < 4:
                    emit_proj(*projs_left.pop(0))
            emit_rest(*jobs[n - 2], state[n - 2][0], state[n - 2][1],
                      state[n - 1][0], ocols[n - 2])
            # bulk sums (all but the final job) ship before the final job
            nc.sync.dma_start(out=sumsd, in_=s_all)
            emit_rest(*jobs[n - 1], state[n - 1][0], state[n - 1][1],
                      state[n - 1][0], ocols[n - 1], fine_dma=True)
            # last job's four sums values go out in a tiny trailing DMA
            nc.sync.dma_start(out=sumsd[:, 4, 4:8], in_=s_all[:, 4, 4:8])

    nc.compile()
    return nc


def _get_nc():
    if "nc" not in _NC_CACHE:
        _NC_CACHE["nc"] = _build_nc()
    return _NC_CACHE["nc"]


def _prep_core(hs_f32, wq_s, wk_s, b, qq):
    """Host-side input prep for core (b, qq). hs_f32: [B, S, D] float32;
    wq_s/wk_s: [D, D] float32 (wq pre-scaled by 1/sqrt(dh))."""
    R0 = ROWS * qq
    g = np.arange(R0 - 7, R0 + ROWS + 7)
    valid = (g >= 0) & (g < S)
    hs_slice = np.zeros((WIN, D), np.float32)
    hs_slice[valid] = hs_f32[b, g[valid]]
    hstw = np.ascontiguousarray(
        hs_slice.T.reshape(4, 128, WIN).transpose(1, 0, 2)).astype(BF)

    hsw = np.zeros((6, 128, D), np.float32)
    starts = [R0 + TOFF[t] - 7 for t in range(5)] + [R0 + 464 - 7]
    for t, st in enumerate(starts):
        g2 = np.arange(st, st + 128)
        v2 = (g2 >= 0) & (g2 < S)
        hsw[t, v2] = hs_f32[b, g2[v2]]
    hsw = hsw.astype(BF)

    slabs = []
    for j in range(4):
        for w in (wq_s, wk_s):  # interleaved: q0,k0,q1,k1,...
            slabs.append(w[:, 128 * j:128 * (j + 1)].reshape(4, 128, 128).transpose(1, 0, 2))
    wpk = np.ascontiguousarray(np.stack(slabs, axis=0)).astype(BF)

    ones = np.ones((128, 8), np.float32)
    if qq == 0:
        ones[0:7, 0] = 0.0
    if qq == 3:
        ones[103:, 4] = 0.0   # tile-4 full window: rows beyond the sequence
        ones[55:, 6] = 0.0    # tile-4 second sub-job window
    ones[:, 5] = 0.0  # zeros column: K=128 preamble for the sums matmuls
    ones = ones.astype(BF)
    return {"hstw": hstw, "wpk": wpk, "hsw": np.ascontiguousarray(hsw), "onesd": ones}


def kernel(hidden_states, Wq, Wk):
    global LAST_RESULTS
    from concourse import bass_utils

    hs_f32 = np.asarray(hidden_states, dtype=np.float32)
    wq_s = np.asarray(Wq, dtype=np.float32) * (1.0 / (DH ** 0.5))
    wk_s = np.asarray(Wk, dtype=np.float32)

    p = np.arange(128)[:, None]
    f = np.arange(128)[None, :]
    negmask = np.where((p - f >= 0) & (p - f <= 14), 0.0, -10000.0).astype(np.float32)
    maskd = np.concatenate([np.eye(128, dtype=np.float32), negmask], axis=1).astype(BF)

    in_maps = []
    for c in range(NCORES):
        m = _prep_core(hs_f32, wq_s, wk_s, c // 4, c % 4)
        m["maskd"] = maskd
        in_maps.append(m)

    nc = _get_nc()
    res = bass_utils.run_bass_kernel_spmd(
        nc, in_maps, core_ids=list(range(NCORES)), trace=TRACE,
    )
    LAST_RESULTS = res

    out = np.empty((2, H, S, D), np.float32)
    jobs = [(TOFF[t], T_LIST[t], t, 0) for t in range(5)] + \
           [(TOFF[t], T_LIST[t], t, 1) for t in range(5)]
    for c in range(NCORES):
        b, qq = c // 4, c % 4
        R0 = ROWS * qq
        flat = np.asarray(res.results[c]["out"]).astype(np.float32)  # [128, 16384]
        sums = np.asarray(res.results[c]["sums"])                    # [128, 6, 8] f32
        off = 0
        for (toff, T, scol, half) in jobs:
            blk = flat[:, off:off + 16 * T].reshape(128, 2, 2, 4, T)
            off += 16 * T
            # blk[p, pair, hh2, dc, i] -> ctx[pair, hh2, i, dc*128+p]
            ctx = blk.transpose(1, 2, 4, 3, 0).reshape(2, 2, T, D)
            rows = slice(R0 + toff, R0 + toff + T)
            for pair in range(2):
                for hh2 in range(2):
                    h = 4 * half + 2 * pair + hh2
                    sm = sums[0:T, scol, 4 * half + 2 * pair + hh2]
                    out[b, h, rows, :] = ctx[pair, hh2] / sm[:, None]
    return out
